# revision 18
# baseline (speedup 1.0000x reference)
"""Trainium2 Bass kernel for nn_DiTBlock (HGRN-attention DiT block).

Sharding: 8 cores = 4 batches x 2 half-sequences (1024 tokens each).
All matmuls run as exact-integer bf16 matmuls (activations quantized to
int8-range integers stored in bf16; ternary weights quantized on host).
Per-token-tile software pipelining keeps the PE queue dense; adaln is
computed distributed (768 outputs/core) with the AllGather overlapped
with the LN-stats pass; the h-carry crosses cores via a pairwise
AllGather overlapped with the g-projection matmuls.
"""
import functools
import numpy as np
import ml_dtypes

import concourse.bass as bass
import concourse.bacc as bacc_mod
import concourse.mybir as mybir
import concourse.tile as tile
from concourse.masks import make_identity
from concourse.bass_utils import run_bass_kernel_spmd

BF16 = ml_dtypes.bfloat16
F32 = mybir.dt.float32
F16 = mybir.dt.float16
BF = mybir.dt.bfloat16
U32 = mybir.dt.uint32
AL = mybir.AluOpType
AF = mybir.ActivationFunctionType
AX = mybir.AxisListType

B, T, D = 4, 2048, 1024
TOK = 1024          # tokens per core
NH, HD = 16, 64
MLP = 4096
N_CORES = 8
C_MAGIC = float(1.5 * 2 ** 23)
MAGIC_U32 = 0x5F3759DF
CAMW = 256          # carry-correction window (f^256 << 1e-6 always)


def _quant_w(w):
    invws = float(np.clip(np.abs(w).mean(dtype=np.float64), 1e-5, None))
    m = np.clip(np.round(w.astype(np.float64) / invws), -1, 1).astype(np.float32)
    return np.ascontiguousarray(m.astype(BF16)), np.float32(invws)


def _build(iw):
    nc = bacc_mod.Bacc("TRN2", target_bir_lowering=False)

    x_sl = nc.declare_dram_parameter("x_sl", [TOK, D], F32, isOutput=False)
    c_cols = nc.declare_dram_parameter("c_cols", [128, 8, B], F32, isOutput=False)
    adw_sl = nc.declare_dram_parameter("adw_sl", [D, 768], F32, isOutput=False)
    adb8 = nc.declare_dram_parameter("adb8", [8, 768], F32, isOutput=False)
    bmask_all = nc.declare_dram_parameter("bmask_all", [32, 8], F32, isOutput=False)
    mask8 = nc.declare_dram_parameter("mask8", [8, 1], F32, isOutput=False)
    gnr = nc.declare_dram_parameter("gnr", [1, D], F32, isOutput=False)
    wiT = nc.declare_dram_parameter("wiT", [D, D], BF, isOutput=False)
    wfT = nc.declare_dram_parameter("wfT", [D, D], BF, isOutput=False)
    wgT = nc.declare_dram_parameter("wgT", [D, D], BF, isOutput=False)
    woT = nc.declare_dram_parameter("woT", [D, D], BF, isOutput=False)
    gwT = nc.declare_dram_parameter("gwT", [D, 2 * MLP], BF, isOutput=False)
    dwT = nc.declare_dram_parameter("dwT", [MLP, D], BF, isOutput=False)
    out_sl = nc.declare_dram_parameter("out_sl", [TOK, D], F32, isOutput=True)

    cc1_in = nc.dram_tensor("cc1_in", [B, 768], F32)
    cc1_out = nc.dram_tensor("cc1_out", [N_CORES * B, 768], F32,
                             addr_space="Shared")
    cc2_in = nc.dram_tensor("cc2_in", [D], F32)
    cc2_out = nc.dram_tensor("cc2_out", [N_CORES, D], F32,
                             addr_space="Shared")

    RG1 = [list(range(N_CORES))]

    with tile.TileContext(nc) as tc:
        import contextlib
        es = contextlib.ExitStack()
        with es:
            cst = es.enter_context(tc.tile_pool(name="cst", bufs=1))
            ps = es.enter_context(tc.tile_pool(name="ps", bufs=1, space="PSUM"))
            dr = es.enter_context(tc.tile_pool(name="dr", bufs=1, space="DRAM"))

            def pmm(tag="mm"):
                return ps.tile([128, 512], F32, tag=tag, name=tag, bufs=4)

            def pbc(shape=(128, 512), dt=F32):
                return ps.tile(list(shape), dt, tag="bc", name="bc", bufs=2)

            # ---------------- consts ----------------
            identb = cst.tile([128, 128], BF, tag="identb")
            make_identity(nc, identb)
            identf = cst.tile([128, 128], F32, tag="identf")
            make_identity(nc, identf)
            ones_row = cst.tile([1, 128], F32, tag="ones_row")
            nc.vector.memset(ones_row, 1.0)
            oi = cst.tile([1, 128], F32, tag="oi")
            nc.vector.memset(oi, float(iw["i"]))
            of = cst.tile([1, 128], F32, tag="of")
            nc.vector.memset(of, float(iw["f"]))
            magic1 = cst.tile([128, 1], F32, tag="magic1")
            nc.vector.memset(magic1[:].bitcast(U32), MAGIC_U32)
            magic16 = cst.tile([128, 16], F32, tag="magic16")
            nc.vector.memset(magic16[:].bitcast(U32), MAGIC_U32)
            mask_sb = cst.tile([8, 1], F32, tag="mask8")
            nc.sync.dma_start(out=mask_sb, in_=mask8[:, :])
            prowc = cst.tile([1, D], F32, tag="prowc")
            dqAg = cst.tile([128, 8], F32, tag="dqAg")
            dqOo = cst.tile([128, 8], F32, tag="dqOo")
            dqCg = cst.tile([128, 8], F32, tag="dqCg")
            dqD8 = cst.tile([128, 8], F32, tag="dqD8")
            lc = cst.tile([128, 8], F32, tag="lc")
            xnew_d = dr.tile([TOK, D], F32, tag="xnew")
            prow_d = dr.tile([6 * D], F32, tag="prow")

            def newton2(pool, x_ap, scale, bias, shape, magic, tag):
                """y = rsqrt(x*scale + bias), 2 Newton iterations."""
                t = pool.tile(shape, F32, tag=tag + "t", name=tag + "t")
                nc.vector.tensor_scalar(out=t, in0=x_ap, scalar1=float(scale),
                                        scalar2=float(bias), op0=AL.mult,
                                        op1=AL.add)
                y = pool.tile(shape, F32, tag=tag + "y", name=tag + "y")
                sh = pool.tile(shape, F32, tag=tag + "s", name=tag + "s")
                nc.vector.tensor_scalar(out=sh[:].bitcast(U32),
                                        in0=t[:].bitcast(U32), scalar1=1,
                                        scalar2=None,
                                        op0=AL.logical_shift_right)
                nc.vector.tensor_tensor(out=y[:].bitcast(U32),
                                        in0=magic[:].bitcast(U32),
                                        in1=sh[:].bitcast(U32),
                                        op=AL.subtract)
                e = pool.tile(shape, F32, tag=tag + "e", name=tag + "e")
                for _ in range(2):
                    nc.vector.tensor_tensor(out=e, in0=y, in1=y, op=AL.mult)
                    nc.vector.tensor_tensor(out=e, in0=e, in1=t, op=AL.mult)
                    nc.vector.tensor_scalar(out=e, in0=e, scalar1=-0.5,
                                            scalar2=1.5, op0=AL.mult,
                                            op1=AL.add)
                    nc.vector.tensor_tensor(out=y, in0=y, in1=e, op=AL.mult)
                return y

            def quant_scales(pool, am, ssq, dk, tag):
                amc = pool.tile([128, 1], F32, tag=tag + "amc", name=tag + "amc")
                nc.vector.tensor_scalar(out=amc, in0=am, scalar1=1e-5,
                                        scalar2=None, op0=AL.max)
                rec = pool.tile([128, 1], F32, tag=tag + "rec", name=tag + "rec")
                nc.vector.reciprocal(out=rec, in_=amc)
                q127 = pool.tile([128, 1], F32, tag=tag + "q", name=tag + "q")
                nc.vector.tensor_scalar(out=q127, in0=rec, scalar1=127.0,
                                        scalar2=None, op0=AL.mult)
                rs = newton2(pool, ssq, 1.0 / dk, 1e-8, [128, 1], magic1,
                             tag + "n")
                dq = pool.tile([128, 1], F32, tag=tag + "dq", name=tag + "dq")
                nc.vector.tensor_tensor(out=dq, in0=amc, in1=rs, op=AL.mult)
                nc.vector.tensor_scalar(out=dq, in0=dq, scalar1=1.0 / 127.0,
                                        scalar2=None, op0=AL.mult)
                return q127, dq

            def round_transpose(pool, src, q_ap, dst_bf, t_idx, nblk, tag):
                """round(src*q) -> bf16 ints, transpose 128-blocks into
                dst_bf[:, g4+jj, 128*t_idx:...]."""
                w = 128 * nblk
                t2 = pool.tile([128, w], F32, tag=tag + "t2", name=tag + "t2")
                nc.scalar.activation(out=t2, in_=src, func=AF.Copy,
                                     scale=q_ap, bias=C_MAGIC)
                kq = pool.tile([128, w], BF, tag=tag + "kq", name=tag + "kq")
                nc.scalar.activation(out=kq, in_=t2, func=AF.Copy,
                                     bias=-C_MAGIC)
                for g4 in range(0, nblk, 4):
                    nb = min(4, nblk - g4)
                    tp = ps.tile([128, 512], BF, tag="tp", name="tp", bufs=2)
                    for jj in range(nb):
                        nc.tensor.transpose(
                            tp[:, 128 * jj:128 * (jj + 1)],
                            kq[:, 128 * (g4 + jj):128 * (g4 + jj + 1)],
                            identb)
                    for jj in range(nb):
                        dst = dst_bf[:, g4 + jj, 128 * t_idx:128 * (t_idx + 1)]
                        if jj % 2 == 0:
                            nc.scalar.copy(out=dst,
                                           in_=tp[:, 128 * jj:128 * (jj + 1)])
                        else:
                            nc.vector.tensor_copy(
                                out=dst, in_=tp[:, 128 * jj:128 * (jj + 1)])

            def bcast_row(pool, row_ap, bname, plus1=False, lhs=None):
                tt = pool.tile([128, D], F32, tag=bname, name=bname, bufs=1)
                for ch in range(0, D, 512):
                    pb = pbc()
                    nc.tensor.matmul(pb, ones_row if lhs is None else lhs,
                                     row_ap[:, ch:ch + 512], start=True,
                                     stop=True)
                    if plus1:
                        nc.scalar.activation(out=tt[:, ch:ch + 512], in_=pb,
                                             func=AF.Identity, bias=1.0)
                    else:
                        nc.scalar.copy(out=tt[:, ch:ch + 512], in_=pb)
                return tt

            def bcast_prow(pool, pidx, bname, plus1=False):
                """Broadcast adaln param row pidx (from prow_d) to [128, D]."""
                nc.scalar.dma_start(
                    out=prowc,
                    in_=prow_d[D * pidx:D * (pidx + 1)].rearrange(
                        "(one d) -> one d", one=1))
                return bcast_row(pool, prowc, bname, plus1=plus1)

            pCM = tc.alloc_tile_pool(name="pCM", bufs=1)
            x2qT = pCM.tile([128, 8, D], BF, tag="x2qT", name="x2qT")
            pWO = tc.alloc_tile_pool(name="pWO", bufs=1)
            wg_sb = pWO.tile([128, 8, D], BF, tag="wg", name="wg")
            wo_sb = pWO.tile([128, 8, D], BF, tag="wo", name="wo")
            xqT = pWO.tile([128, 8, D], BF, tag="xqT", name="xqT")
            gnr_sb = pWO.tile([1, D], F32, tag="gnr")
            nc.sync.dma_start(out=gnr_sb, in_=gnr[:, :])
            pHA = tc.alloc_tile_pool(name="pHA", bufs=1)
            ha = pHA.tile([128, 8, TOK], F32, tag="ha", name="ha")
            cam = pHA.tile([128, 8, CAMW], BF, tag="cam", name="cam")
            pL2 = tc.alloc_tile_pool(name="pL2", bufs=1)
            wi_sb = pL2.tile([128, 8, D], BF, tag="wi", name="wi")
            wf_sb = pL2.tile([128, 8, D], BF, tag="wf", name="wf")
            dqrow = pL2.tile([1, D], F32, tag="dqrow")
            Sb_i = pL2.tile([128, D], F32, tag="Sbi")
            Sb_f = pL2.tile([128, D], F32, tag="Sbf")

            # ---------------- prologue: prefetch + adaln ----------------
            pP = tc.alloc_tile_pool(name="pP", bufs=1)
            c_sb = pP.tile([128, 8, B], F32, tag="csb")
            nc.sync.dma_start(out=c_sb, in_=c_cols[:, :, :])
            adw_ch = []
            for jh in range(2):
                adw_h = pP.tile([128, 4, 768], F32, tag="adw", bufs=2)
                for jq in range(4):
                    nc.sync.dma_start(
                        out=adw_h[:, jq, :],
                        in_=adw_sl[512 * jh + 128 * jq:
                                   512 * jh + 128 * (jq + 1), :].rearrange(
                            "(a p) q -> p a q", p=128)[:, 0, :])
                adw_ch.append(adw_h)
            for (wsb, wdr) in ((wi_sb, wiT), (wf_sb, wfT)):
                for jh in range(2):
                    nc.gpsimd.dma_start(
                        out=wsb[:, 4 * jh:4 * (jh + 1), :],
                        in_=wdr[512 * jh:512 * (jh + 1), :].rearrange(
                            "(a p) q -> p a q", p=128))
            adb_sb = pP.tile([8, 768], F32, tag="adb")
            nc.sync.dma_start(out=adb_sb, in_=adb8[:, :])
            bm_sb = pP.tile([32, 8], F32, tag="bm")
            nc.sync.dma_start(out=bm_sb, in_=bmask_all[:, :])
            for (wsb, wdr) in ((wg_sb, wgT), (wo_sb, woT)):
                for jh in range(2):
                    nc.gpsimd.dma_start(
                        out=wsb[:, 4 * jh:4 * (jh + 1), :],
                        in_=wdr[512 * jh:512 * (jh + 1), :].rearrange(
                            "(a p) q -> p a q", p=128))

            # silu(c) = c * sigmoid(c) (keeps the sigmoid act table)
            csg = pP.tile([128, 8, B], F32, tag="csg")
            nc.scalar.activation(out=csg, in_=c_sb, func=AF.Sigmoid)
            cs_sb = pP.tile([128, 8, B], F32, tag="cssb")
            nc.vector.tensor_tensor(out=cs_sb, in0=c_sb, in1=csg, op=AL.mult)
            psA = pbc((B, 512))
            psB = pbc((B, 256))
            for j in range(8):
                aw = adw_ch[j // 4][:, j % 4, :]
                nc.tensor.matmul(psA, cs_sb[:, j, :], aw[:, 0:512],
                                 start=(j == 0), stop=(j == 7))
                nc.tensor.matmul(psB, cs_sb[:, j, :], aw[:, 512:768],
                                 start=(j == 0), stop=(j == 7))
            ad_sb = pP.tile([B, 768], F32, tag="adsb")
            nc.scalar.copy(out=ad_sb[:, 0:512], in_=psA)
            nc.scalar.copy(out=ad_sb[:, 512:768], in_=psB)
            nc.sync.dma_start(out=cc1_in[:, :], in_=ad_sb)
            nc.gpsimd.collective_compute(
                "AllGather", AL.bypass, ins=[cc1_in[:]], outs=[cc1_out[:]],
                replica_groups=RG1)

            # ---- param extraction: emitted early, runs when cc1 lands.
            # scalar/gpsimd queues so A1's sync-DMAs + DVE work don't block.
            ag_all = pP.tile([32, 768], F32, tag="agall", bufs=1)
            nc.scalar.dma_start(out=ag_all, in_=cc1_out[:, :])
            ps1 = pbc((8, 512))
            ps2 = pbc((8, 256))
            nc.tensor.matmul(ps1, bm_sb, ag_all[:, 0:512],
                             start=True, stop=True)
            nc.tensor.matmul(ps2, bm_sb, ag_all[:, 512:768],
                             start=True, stop=True)
            sel_raw = pP.tile([8, 768], F32, tag="selr", bufs=1)
            nc.scalar.copy(out=sel_raw[:, 0:512], in_=ps1)
            nc.scalar.copy(out=sel_raw[:, 512:768], in_=ps2)
            sel_sb = pP.tile([8, 768], F32, tag="sel", bufs=1)
            nc.gpsimd.tensor_tensor(out=sel_sb, in0=sel_raw, in1=adb_sb,
                                    op=AL.add)
            nc.scalar.dma_start(
                out=prow_d[:].rearrange("(a q) -> a q", a=8), in_=sel_sb)
            pP.release()

            # ======== phase A: LN1 (overlaps cc1) + modulate + quant ======
            rstdA = cst.tile([128, 8], F32, tag="rstdA")
            nmrA = cst.tile([128, 8], F32, tag="nmrA")
            with tc.tile_pool(name="pA", bufs=2) as pA:
                for t in range(8):
                    xt = pA.tile([128, D], F32, tag="xt", bufs=2)
                    nc.sync.dma_start(out=xt,
                                      in_=x_sl[128 * t:128 * (t + 1), :])
                    st = pA.tile([128, 2, 6], F32, tag="bst")
                    xr = xt.rearrange("p (s d) -> p s d", s=2)
                    for s2 in range(2):
                        nc.vector.bn_stats(out=st[:, s2, :], in_=xr[:, s2, :])
                    mv = pA.tile([128, 2], F32, tag="bmv")
                    nc.vector.bn_aggr(out=mv, in_=st)
                    rst = newton2(pA, mv[:, 1:2], 1.0, 1e-6, [128, 1],
                                  magic1, "rA")
                    nc.vector.tensor_copy(out=rstdA[:, t:t + 1], in_=rst)
                    nm = pA.tile([128, 1], F32, tag="nmA")
                    nc.vector.tensor_tensor(out=nm, in0=mv[:, 0:1], in1=rst,
                                            op=AL.mult)
                    nc.vector.tensor_scalar(out=nmrA[:, t:t + 1], in0=nm,
                                            scalar1=-1.0, scalar2=None,
                                            op0=AL.mult)

                B_sh1 = bcast_prow(pA, 0, "Bsh1")
                B_sc1 = bcast_prow(pA, 1, "Bsc1", plus1=True)

                # ---- modulate + quant + round + transpose, per tile ----
                for t in range(8):
                    xt = pA.tile([128, D], F32, tag="xt", bufs=2)
                    nc.sync.dma_start(out=xt,
                                      in_=x_sl[128 * t:128 * (t + 1), :])
                    mo = pA.tile([128, D], F32, tag="mo", bufs=2)
                    nc.scalar.activation(out=mo, in_=xt, func=AF.Identity,
                                         scale=rstdA[:, t:t + 1],
                                         bias=nmrA[:, t:t + 1])
                    nc.gpsimd.tensor_tensor(out=mo, in0=mo, in1=B_sc1,
                                            op=AL.mult)
                    nc.gpsimd.tensor_tensor(out=mo, in0=mo, in1=B_sh1,
                                            op=AL.add)
                    am = pA.tile([128, 1], F32, tag="amA")
                    nc.vector.tensor_reduce(out=am, in_=mo, axis=AX.X,
                                            op=AL.max,
                                            apply_absolute_value=True)
                    ssq = pA.tile([128, 1], F32, tag="ssA")
                    scr = pA.tile([128, D], F32, tag="rAt2", bufs=2)
                    nc.scalar.activation(out=scr, in_=mo, func=AF.Square,
                                         accum_out=ssq)
                    q127, dq = quant_scales(pA, am, ssq, D, "qA")
                    nc.vector.tensor_scalar(out=dqAg[:, t:t + 1], in0=dq,
                                            scalar1=float(iw["g"]),
                                            scalar2=None, op0=AL.mult)
                    pdq = pbc((1, 128))
                    nc.tensor.transpose(pdq, dq, identf)
                    nc.vector.tensor_copy(
                        out=dqrow[:, 128 * t:128 * (t + 1)], in_=pdq)
                    round_transpose(pA, mo, q127, xqT, t, 8, "rA")
                    if t == 3 or t == 7:
                        ch = 0 if t == 3 else 512
                        pb = pbc()
                        nc.tensor.matmul(pb, oi, dqrow[:, ch:ch + 512],
                                         start=True, stop=True)
                        nc.scalar.copy(out=Sb_i[:, ch:ch + 512], in_=pb)
                        pb2 = pbc()
                        nc.tensor.matmul(pb2, of, dqrow[:, ch:ch + 512],
                                         start=True, stop=True)
                        nc.scalar.copy(out=Sb_f[:, ch:ch + 512], in_=pb2)

            # ======== phase B: i/f matmuls + scan ========
            with tc.tile_pool(name="pB", bufs=2) as pB:
                for m in range(8):
                    ft = pB.tile([128, TOK], F32, tag="ft", bufs=2)
                    it = pB.tile([128, TOK], F32, tag="it", bufs=2)
                    for ck in (0, 512):
                        pf = pmm()
                        for j in range(8):
                            nc.tensor.matmul(pf, wf_sb[:, j, 128 * m:128 * (m + 1)],
                                             xqT[:, j, ck:ck + 512],
                                             start=(j == 0), stop=(j == 7))
                        pi = pmm()
                        for j in range(8):
                            nc.tensor.matmul(pi, wi_sb[:, j, 128 * m:128 * (m + 1)],
                                             xqT[:, j, ck:ck + 512],
                                             start=(j == 0), stop=(j == 7))
                        nc.vector.tensor_tensor(out=ft[:, ck:ck + 512], in0=pf,
                                                in1=Sb_f[:, ck:ck + 512],
                                                op=AL.mult)
                        nc.vector.tensor_tensor(out=it[:, ck:ck + 512], in0=pi,
                                                in1=Sb_i[:, ck:ck + 512],
                                                op=AL.mult)
                    sigf = pB.tile([128, TOK], F32, tag="sigf", bufs=2)
                    nc.scalar.activation(out=sigf, in_=ft, func=AF.Sigmoid)
                    sgi = pB.tile([128, TOK], F32, tag="sgi", bufs=2)
                    nc.scalar.activation(out=sgi, in_=it, func=AF.Sigmoid)
                    # omf = 1 - sigf (in place over ft); ifin = it*sgi*omf
                    nc.vector.tensor_scalar(out=ft, in0=sigf, scalar1=-1.0,
                                            scalar2=1.0, op0=AL.mult,
                                            op1=AL.add)
                    nc.gpsimd.tensor_tensor(out=it, in0=it, in1=sgi,
                                            op=AL.mult)
                    nc.gpsimd.tensor_tensor(out=it, in0=it, in1=ft,
                                            op=AL.mult)
                    nc.vector.tensor_tensor_scan(ha[:, m, :], sigf, it, 0.0,
                                                 op0=AL.mult, op1=AL.add)
                    nc.vector.tensor_tensor_scan(cam[:, m, :],
                                                 sigf[:, 0:CAMW],
                                                 sigf[:, 0:CAMW], 1.0,
                                                 op0=AL.mult, op1=AL.bypass)
                    nc.vector.tensor_copy(out=lc[:, m:m + 1],
                                          in_=ha[:, m, TOK - 1:TOK])
                plc = pbc((8, 128))
                nc.tensor.transpose(plc, lc, identf)
                lcT = pB.tile([8, 128], F32, tag="lcT", bufs=1)
                nc.vector.tensor_copy(out=lcT, in_=plc)
                nc.sync.dma_start(
                    out=cc2_in[:].rearrange("(a q) -> a q", a=8), in_=lcT)
                nc.gpsimd.collective_compute(
                    "AllGather", AL.bypass, ins=[cc2_in[:]], outs=[cc2_out[:]],
                    replica_groups=RG1)

            pL2.release()

            # ======== phase OC: per-tile attention-out + residual + LN2 ===
            # tile order [2..7, 0, 1]: tiles >= 2 are outside the carry
            # window so they proceed while the AllGather is in flight.
            with tc.tile_pool(name="pOC", bufs=2) as pOC:
                B_gn = bcast_row(pOC, gnr_sb, "Bgn")
                B_g1 = bcast_prow(pOC, 2, "Bg1")
                B_sh2 = bcast_prow(pOC, 3, "Bsh2")
                B_sc2 = bcast_prow(pOC, 4, "Bsc2", plus1=True)
                carr = pOC.tile([128, 8], F32, tag="carr", bufs=1)
                for t in [2, 3, 4, 5, 6, 7, 0, 1]:
                    if t == 0:
                        # consume cc2: carry per channel block, then fix the
                        # first CAMW tokens of ha
                        ag2 = pOC.tile([N_CORES, D], F32, tag="ag2", bufs=1)
                        nc.sync.dma_start(out=ag2, in_=cc2_out[:, :])
                        for m in range(8):
                            pc = pbc((128, 1))
                            nc.tensor.matmul(pc, ag2[:, 128 * m:128 * (m + 1)],
                                             mask_sb, start=True, stop=True)
                            nc.vector.tensor_copy(out=carr[:, m:m + 1],
                                                  in_=pc)
                        for m in range(8):
                            nc.vector.scalar_tensor_tensor(
                                out=ha[:, m, 0:CAMW], in0=cam[:, m, :],
                                scalar=carr[:, m:m + 1], in1=ha[:, m, 0:CAMW],
                                op0=AL.mult, op1=AL.add)
                    # transpose h column-block t (all m) into hT_t
                    hTt = pOC.tile([128, D], F32, tag="hTt", bufs=2)
                    for g4 in range(0, 8, 4):
                        tpf = pbc()
                        for jj in range(4):
                            m = g4 + jj
                            nc.tensor.transpose(
                                tpf[:, 128 * jj:128 * (jj + 1)],
                                ha[:, m, 128 * t:128 * (t + 1)], identf)
                        for jj in range(4):
                            m = g4 + jj
                            dst = hTt[:, 128 * m:128 * (m + 1)]
                            if jj % 2 == 0:
                                nc.scalar.copy(
                                    out=dst,
                                    in_=tpf[:, 128 * jj:128 * (jj + 1)])
                            else:
                                nc.vector.tensor_copy(
                                    out=dst,
                                    in_=tpf[:, 128 * jj:128 * (jj + 1)])
                    # g projection for this tile
                    gst = pOC.tile([128, D], F32, tag="gst", bufs=2)
                    for ck in (0, 512):
                        pg = pmm()
                        for j in range(8):
                            nc.tensor.matmul(pg, xqT[:, j, 128 * t:128 * (t + 1)],
                                             wg_sb[:, j, ck:ck + 512],
                                             start=(j == 0), stop=(j == 7))
                        sil = pOC.tile([128, 512], F32, tag="silg", bufs=2)
                        nc.scalar.activation(out=sil, in_=pg, func=AF.Silu,
                                             scale=dqAg[:, t:t + 1])
                        nc.vector.tensor_tensor(out=gst[:, ck:ck + 512],
                                                in0=sil,
                                                in1=B_gn[:, ck:ck + 512],
                                                op=AL.mult)
                    # GN + gate + quantize o
                    sq = pOC.tile([128, D], F32, tag="big", bufs=2)
                    nc.gpsimd.tensor_tensor(out=sq, in0=hTt, in1=hTt,
                                            op=AL.mult)
                    msh = pOC.tile([128, NH], F32, tag="msh", bufs=2)
                    nc.vector.tensor_reduce(
                        out=msh, in_=sq.rearrange("p (h d) -> p h d", h=NH),
                        axis=AX.X, op=AL.add)
                    rsH = newton2(pOC, msh, 1.0 / HD, 1e-5, [128, NH],
                                  magic16, "rH")
                    rb = bass.AP(tensor=rsH.tensor, offset=rsH.offset,
                                 ap=[rsH.ap[0], [1, NH], [0, HD]])
                    oa = pOC.tile([128, D], F32, tag="big", bufs=2)
                    nc.gpsimd.tensor_tensor(
                        out=oa.rearrange("p (h d) -> p h d", h=NH),
                        in0=hTt.rearrange("p (h d) -> p h d", h=NH),
                        in1=rb, op=AL.mult)
                    nc.vector.tensor_tensor(out=oa, in0=oa, in1=gst,
                                            op=AL.mult)
                    am = pOC.tile([128, 1], F32, tag="amO")
                    nc.vector.tensor_reduce(out=am, in_=oa, axis=AX.X,
                                            op=AL.max,
                                            apply_absolute_value=True)
                    ssq = pOC.tile([128, 1], F32, tag="ssO")
                    scr = pOC.tile([128, D], F32, tag="rtt2", bufs=2)
                    nc.scalar.activation(out=scr, in_=oa, func=AF.Square,
                                         accum_out=ssq)
                    q127, dq = quant_scales(pOC, am, ssq, D, "qO")
                    dqO = pOC.tile([128, 1], F32, tag="dqO", bufs=2)
                    nc.vector.tensor_scalar(out=dqO, in0=dq,
                                            scalar1=float(iw["o"]),
                                            scalar2=None, op0=AL.mult)
                    oqTt = pOC.tile([128, 8, 128], BF, tag="oqTt", bufs=2)
                    round_transpose(pOC, oa, q127, oqTt, 0, 8, "rt")
                    # wo matmul + residual + LN2 + modulate + quantize
                    x2 = pOC.tile([128, D], F32, tag="x2", bufs=2)
                    nc.sync.dma_start(out=x2,
                                      in_=x_sl[128 * t:128 * (t + 1), :])
                    xn = pOC.tile([128, D], F32, tag="xn", bufs=2)
                    for ck in (0, 512):
                        at = pmm()
                        for j in range(8):
                            nc.tensor.matmul(at, oqTt[:, j, :],
                                             wo_sb[:, j, ck:ck + 512],
                                             start=(j == 0), stop=(j == 7))
                        ga = pOC.tile([128, 512], F32, tag="ga", bufs=2)
                        nc.vector.scalar_tensor_tensor(
                            out=ga, in0=at, scalar=dqO,
                            in1=B_g1[:, ck:ck + 512], op0=AL.mult,
                            op1=AL.mult)
                        nc.gpsimd.tensor_tensor(out=xn[:, ck:ck + 512],
                                                in0=ga, in1=x2[:, ck:ck + 512],
                                                op=AL.add)
                    nc.gpsimd.dma_start(out=xnew_d[128 * t:128 * (t + 1), :],
                                         in_=xn)
                    st = pOC.tile([128, 2, 6], F32, tag="bst2")
                    xr = xn.rearrange("p (s d) -> p s d", s=2)
                    for s2 in range(2):
                        nc.vector.bn_stats(out=st[:, s2, :], in_=xr[:, s2, :])
                    mv = pOC.tile([128, 2], F32, tag="bmv2")
                    nc.vector.bn_aggr(out=mv, in_=st)
                    rst = newton2(pOC, mv[:, 1:2], 1.0, 1e-6, [128, 1],
                                  magic1, "rC")
                    nm = pOC.tile([128, 1], F32, tag="nmC")
                    nc.vector.tensor_tensor(out=nm, in0=mv[:, 0:1], in1=rst,
                                            op=AL.mult)
                    nc.vector.tensor_scalar(out=nm, in0=nm, scalar1=-1.0,
                                            scalar2=None, op0=AL.mult)
                    u2 = pOC.tile([128, D], F32, tag="u2", bufs=2)
                    nc.scalar.activation(out=u2, in_=xn, func=AF.Identity,
                                         scale=rst, bias=nm)
                    nc.vector.tensor_tensor(out=u2, in0=u2, in1=B_sc2,
                                            op=AL.mult)
                    nc.gpsimd.tensor_tensor(out=u2, in0=u2, in1=B_sh2,
                                            op=AL.add)
                    am2 = pOC.tile([128, 1], F32, tag="amC")
                    nc.vector.tensor_reduce(out=am2, in_=u2, axis=AX.X,
                                            op=AL.max,
                                            apply_absolute_value=True)
                    ssq2 = pOC.tile([128, 1], F32, tag="ssC")
                    scr2 = pOC.tile([128, D], F32, tag="rtt2", bufs=2)
                    nc.scalar.activation(out=scr2, in_=u2, func=AF.Square,
                                         accum_out=ssq2)
                    q127c, dqc = quant_scales(pOC, am2, ssq2, D, "qC")
                    nc.vector.tensor_scalar(out=dqCg[:, t:t + 1], in0=dqc,
                                            scalar1=float(iw["gate"]),
                                            scalar2=None, op0=AL.mult)
                    round_transpose(pOC, u2, q127c, x2qT, t, 8, "rt")

            pHA.release()
            pWO.release()

            # ======== phase D/E: MLP in two 512-token supertiles ========
            with tc.tile_pool(name="pM", bufs=2) as pM:
                B_g2 = bcast_prow(pM, 5, "Bg2")
                dw_sb = pM.tile([128, 32, D], BF, tag="dw", bufs=1)
                h2 = pM.tile([128, 4, MLP], F16, tag="h2", bufs=1)
                amDg = pM.tile([128, 4, 8], F32, tag="amDg", bufs=1)
                ssDg = pM.tile([128, 4, 8], F32, tag="ssDg", bufs=1)
                for s in range(2):
                    for g in range(8):
                        gwg = pM.tile([128, 8, 512], BF, tag="gwg", bufs=2)
                        gwy = pM.tile([128, 8, 512], BF, tag="gwy", bufs=2)
                        for ah in range(4):
                            nc.gpsimd.dma_start(
                                out=gwg[:, 2 * ah:2 * (ah + 1), :],
                                in_=gwT[256 * ah:256 * (ah + 1),
                                        512 * g:512 * (g + 1)].rearrange(
                                    "(a p) q -> p a q", p=128))
                            nc.gpsimd.dma_start(
                                out=gwy[:, 2 * ah:2 * (ah + 1), :],
                                in_=gwT[256 * ah:256 * (ah + 1),
                                        MLP + 512 * g:MLP + 512 * (g + 1)]
                                .rearrange("(a p) q -> p a q", p=128))
                        for tt in range(4):
                            t = 4 * s + tt
                            pg = pmm()
                            for j in range(8):
                                nc.tensor.matmul(
                                    pg, x2qT[:, j, 128 * t:128 * (t + 1)],
                                    gwg[:, j, :], start=(j == 0),
                                    stop=(j == 7))
                            py = pmm()
                            for j in range(8):
                                nc.tensor.matmul(
                                    py, x2qT[:, j, 128 * t:128 * (t + 1)],
                                    gwy[:, j, :], start=(j == 0),
                                    stop=(j == 7))
                            sil = pM.tile([128, 512], F32, tag="silm", bufs=2)
                            nc.scalar.activation(out=sil, in_=pg,
                                                 func=AF.Silu,
                                                 scale=dqCg[:, t:t + 1])
                            h2s = h2[:, tt, 512 * g:512 * (g + 1)]
                            nc.vector.tensor_tensor(out=h2s, in0=sil, in1=py,
                                                    op=AL.mult)
                            nc.vector.tensor_reduce(
                                out=amDg[:, tt, g:g + 1], in_=h2s, axis=AX.X,
                                op=AL.max, apply_absolute_value=True)
                            scr = pM.tile([128, 512], F32, tag="sqD", bufs=1)
                            nc.scalar.activation(
                                out=scr, in_=h2s, func=AF.Square,
                                accum_out=ssDg[:, tt, g:g + 1])
                        if s == 0:
                            nc.gpsimd.dma_start(
                                out=dw_sb[:, 4 * g:4 * (g + 1), :],
                                in_=dwT[512 * g:512 * (g + 1), :]
                                .rearrange("(a p) q -> p a q", p=128))
                    # ---- round + transpose + down-proj per token tile ----
                    for tt in range(4):
                        t = 4 * s + tt
                        am = pM.tile([128, 1], F32, tag="amD")
                        nc.vector.tensor_reduce(out=am, in_=amDg[:, tt, :],
                                                axis=AX.X, op=AL.max)
                        ssq = pM.tile([128, 1], F32, tag="ssD")
                        nc.vector.tensor_reduce(out=ssq, in_=ssDg[:, tt, :],
                                                axis=AX.X, op=AL.add)
                        if tt == 0:
                            xn3 = pM.tile([128, D], F32, tag="xn3", bufs=2)
                            nc.sync.dma_start(
                                out=xn3,
                                in_=xnew_d[128 * t:128 * (t + 1), :])
                        else:
                            xn3 = xn3_next
                        if tt < 3:
                            xn3_next = pM.tile([128, D], F32, tag="xn3",
                                               bufs=2)
                            nc.sync.dma_start(
                                out=xn3_next,
                                in_=xnew_d[128 * (t + 1):128 * (t + 2), :])
                        q127, dq = quant_scales(pM, am, ssq, MLP, "qD")
                        nc.vector.tensor_scalar(out=dqD8[:, t:t + 1], in0=dq,
                                                scalar1=float(iw["down"]),
                                                scalar2=None, op0=AL.mult)
                        h2qT = pM.tile([128, 32, 128], BF, tag="h2qT", bufs=2)
                        for qc in range(4):
                            round_transpose(
                                pM, h2[:, tt, 1024 * qc:1024 * (qc + 1)],
                                q127, h2qT[:, 8 * qc:8 * (qc + 1), :],
                                0, 8, "rD")
                        outt = pM.tile([128, D], F32, tag="outt", bufs=2)
                        for ck in (0, 512):
                            pdn = pmm()
                            for j2 in range(32):
                                nc.tensor.matmul(pdn, h2qT[:, j2, :],
                                                 dw_sb[:, j2, ck:ck + 512],
                                                 start=(j2 == 0),
                                                 stop=(j2 == 31))
                            gd = pM.tile([128, 512], F32, tag="gd", bufs=1)
                            nc.vector.scalar_tensor_tensor(
                                out=gd, in0=pdn, scalar=dqD8[:, t:t + 1],
                                in1=B_g2[:, ck:ck + 512], op0=AL.mult,
                                op1=AL.mult)
                            nc.vector.tensor_tensor(
                                out=outt[:, ck:ck + 512], in0=gd,
                                in1=xn3[:, ck:ck + 512], op=AL.add)
                        nc.sync.dma_start(
                            out=out_sl[128 * t:128 * (t + 1), :], in_=outt)

            pCM.release()

    nc.finalize()
    return nc


@functools.lru_cache(maxsize=2)
def _build_cached(iw_items):
    return _build(dict(iw_items))


def kernel(x, c, adaln_w, adaln_b, wi, wf, wg, gnorm_w, wo, gate_w, down_w):
    x = np.ascontiguousarray(np.asarray(x, dtype=np.float32))
    c = np.ascontiguousarray(np.asarray(c, dtype=np.float32))
    adaln_w = np.asarray(adaln_w, dtype=np.float32)
    adaln_b = np.asarray(adaln_b, dtype=np.float32)
    gnorm_w = np.asarray(gnorm_w, dtype=np.float32)

    mi, iwi = _quant_w(np.asarray(wi, dtype=np.float32))
    mf, iwf = _quant_w(np.asarray(wf, dtype=np.float32))
    mg, iwg = _quant_w(np.asarray(wg, dtype=np.float32))
    mo, iwo = _quant_w(np.asarray(wo, dtype=np.float32))
    mgate, iwgate = _quant_w(np.asarray(gate_w, dtype=np.float32))
    mdown, iwdown = _quant_w(np.asarray(down_w, dtype=np.float32))

    iw = {"i": float(iwi), "f": float(iwf), "g": float(iwg), "o": float(iwo),
          "gate": float(iwgate), "down": float(iwdown)}
    nc = _build_cached(tuple(sorted(iw.items())))

    wiT_h = np.ascontiguousarray(mi.T)
    wfT_h = np.ascontiguousarray(mf.T)
    wgT_h = np.ascontiguousarray(mg.T)
    woT_h = np.ascontiguousarray(mo.T)
    gwT_h = np.ascontiguousarray(mgate.T)
    dwT_h = np.ascontiguousarray(mdown.T)
    adwT = np.ascontiguousarray(adaln_w.T)          # [D, 6D] f32
    adb8_h = np.ascontiguousarray(adaln_b.reshape(8, 768))
    gnr_h = np.ascontiguousarray(np.tile(gnorm_w, NH)[None, :])
    c_cols_h = np.ascontiguousarray(
        c.T.reshape(8, 128, B).transpose(1, 0, 2))   # [128, 8, B]

    in_maps = []
    for core in range(N_CORES):
        b, half = core // 2, core % 2
        bmask = np.zeros((32, 8), np.float32)
        for r in range(8):
            bmask[4 * r + b, r] = 1.0
        m8 = np.zeros((8, 1), np.float32)
        if half == 1:
            m8[core - 1, 0] = 1.0

        in_maps.append({
            "x_sl": np.ascontiguousarray(x[b, half * TOK:(half + 1) * TOK, :]),
            "c_cols": c_cols_h,
            "adw_sl": np.ascontiguousarray(
                adwT[:, 768 * core:768 * (core + 1)]),
            "adb8": adb8_h,
            "bmask_all": bmask,
            "mask8": m8,
            "gnr": gnr_h,
            "wiT": wiT_h, "wfT": wfT_h, "wgT": wgT_h, "woT": woT_h,
            "gwT": gwT_h, "dwT": dwT_h,
        })

    res = run_bass_kernel_spmd(nc, in_maps, core_ids=list(range(N_CORES)))
    out = np.zeros((B, T, D), np.float32)
    for core in range(N_CORES):
        b, half = core // 2, core % 2
        out[b, half * TOK:(half + 1) * TOK, :] = res.results[core]["out_sl"]
    return out


# revision 21
# speedup vs baseline: 1.0579x; 1.0579x over previous
"""Trainium2 Bass kernel for nn_DiTBlock (HGRN-attention DiT block).

Sharding: 8 cores = 4 batches x 2 half-sequences (1024 tokens each).
All matmuls run as exact-integer bf16 matmuls (activations quantized to
int8-range integers stored in bf16; ternary weights quantized on host).
Per-token-tile software pipelining keeps the PE queue dense; adaln is
computed distributed (768 outputs/core) with the AllGather overlapped
with the LN-stats pass; the h-carry crosses cores via a pairwise
AllGather overlapped with the g-projection matmuls.
"""
import functools
import numpy as np
import ml_dtypes

import concourse.bass as bass
import concourse.bacc as bacc_mod
import concourse.mybir as mybir
import concourse.tile as tile
from concourse.masks import make_identity
from concourse.bass_utils import run_bass_kernel_spmd

BF16 = ml_dtypes.bfloat16
F32 = mybir.dt.float32
F16 = mybir.dt.float16
BF = mybir.dt.bfloat16
U32 = mybir.dt.uint32
AL = mybir.AluOpType
AF = mybir.ActivationFunctionType
AX = mybir.AxisListType

B, T, D = 4, 2048, 1024
TOK = 1024          # tokens per core
NH, HD = 16, 64
MLP = 4096
N_CORES = 8
C_MAGIC = float(1.5 * 2 ** 23)
MAGIC_U32 = 0x5F3759DF
CAMW = 256          # carry-correction window (f^256 << 1e-6 always)


def _quant_w(w):
    invws = float(np.clip(np.abs(w).mean(dtype=np.float64), 1e-5, None))
    m = np.clip(np.round(w.astype(np.float64) / invws), -1, 1).astype(np.float32)
    return np.ascontiguousarray(m.astype(BF16)), np.float32(invws)


def _build(iw):
    nc = bacc_mod.Bacc("TRN2", target_bir_lowering=False)

    x_sl = nc.declare_dram_parameter("x_sl", [TOK, D], F32, isOutput=False)
    c_cols = nc.declare_dram_parameter("c_cols", [128, 8, B], F32, isOutput=False)
    adw_sl = nc.declare_dram_parameter("adw_sl", [D, 768], F32, isOutput=False)
    adb8 = nc.declare_dram_parameter("adb8", [8, 768], F32, isOutput=False)
    bmask_all = nc.declare_dram_parameter("bmask_all", [32, 8], F32, isOutput=False)
    mask8 = nc.declare_dram_parameter("mask8", [8, 1], F32, isOutput=False)
    gnr = nc.declare_dram_parameter("gnr", [1, D], F32, isOutput=False)
    wiT = nc.declare_dram_parameter("wiT", [D, D], BF, isOutput=False)
    wfT = nc.declare_dram_parameter("wfT", [D, D], BF, isOutput=False)
    wgT = nc.declare_dram_parameter("wgT", [D, D], BF, isOutput=False)
    woT = nc.declare_dram_parameter("woT", [D, D], BF, isOutput=False)
    gwT = nc.declare_dram_parameter("gwT", [D, 2 * MLP], BF, isOutput=False)
    dwT = nc.declare_dram_parameter("dwT", [MLP, D], BF, isOutput=False)
    out_sl = nc.declare_dram_parameter("out_sl", [TOK, D], F32, isOutput=True)

    cc1_in = nc.dram_tensor("cc1_in", [B, 768], F32)
    cc1_out = nc.dram_tensor("cc1_out", [N_CORES * B, 768], F32,
                             addr_space="Shared")
    cc2_in = nc.dram_tensor("cc2_in", [D], F32)
    cc2_out = nc.dram_tensor("cc2_out", [N_CORES, D], F32,
                             addr_space="Shared")

    RG1 = [list(range(N_CORES))]

    with tile.TileContext(nc) as tc:
        import contextlib
        es = contextlib.ExitStack()
        with es:
            cst = es.enter_context(tc.tile_pool(name="cst", bufs=1))
            ps = es.enter_context(tc.tile_pool(name="ps", bufs=1, space="PSUM"))
            dr = es.enter_context(tc.tile_pool(name="dr", bufs=1, space="DRAM"))

            def pmm(tag="mm"):
                return ps.tile([128, 512], F32, tag=tag, name=tag, bufs=4)

            def pbc(shape=(128, 512), dt=F32):
                return ps.tile(list(shape), dt, tag="bc", name="bc", bufs=2)

            # ---------------- consts ----------------
            identb = cst.tile([128, 128], BF, tag="identb")
            make_identity(nc, identb)
            identf = cst.tile([128, 128], F32, tag="identf")
            make_identity(nc, identf)
            ones_row = cst.tile([1, 128], F32, tag="ones_row")
            nc.vector.memset(ones_row, 1.0)
            oi = cst.tile([1, 128], F32, tag="oi")
            nc.vector.memset(oi, float(iw["i"]))
            of = cst.tile([1, 128], F32, tag="of")
            nc.vector.memset(of, float(iw["f"]))
            magic1 = cst.tile([128, 1], F32, tag="magic1")
            nc.vector.memset(magic1[:].bitcast(U32), MAGIC_U32)
            magic16 = cst.tile([128, 16], F32, tag="magic16")
            nc.vector.memset(magic16[:].bitcast(U32), MAGIC_U32)
            mask_sb = cst.tile([8, 1], F32, tag="mask8")
            nc.sync.dma_start(out=mask_sb, in_=mask8[:, :])
            prowc = cst.tile([1, D], F32, tag="prowc")
            dqAg = cst.tile([128, 8], F32, tag="dqAg")
            dqOo = cst.tile([128, 8], F32, tag="dqOo")
            dqCg = cst.tile([128, 8], F32, tag="dqCg")
            dqD8 = cst.tile([128, 8], F32, tag="dqD8")
            lc = cst.tile([128, 8], F32, tag="lc")
            xnew_d = dr.tile([TOK, D], F32, tag="xnew")
            prow_d = dr.tile([6 * D], F32, tag="prow")

            def newton2(pool, x_ap, scale, bias, shape, magic, tag,
                        iters=2):
                """y = rsqrt(x*scale + bias), Newton iterations."""
                t = pool.tile(shape, F32, tag=tag + "t", name=tag + "t")
                nc.vector.tensor_scalar(out=t, in0=x_ap, scalar1=float(scale),
                                        scalar2=float(bias), op0=AL.mult,
                                        op1=AL.add)
                y = pool.tile(shape, F32, tag=tag + "y", name=tag + "y")
                sh = pool.tile(shape, F32, tag=tag + "s", name=tag + "s")
                nc.vector.tensor_scalar(out=sh[:].bitcast(U32),
                                        in0=t[:].bitcast(U32), scalar1=1,
                                        scalar2=None,
                                        op0=AL.logical_shift_right)
                nc.vector.tensor_tensor(out=y[:].bitcast(U32),
                                        in0=magic[:].bitcast(U32),
                                        in1=sh[:].bitcast(U32),
                                        op=AL.subtract)
                e = pool.tile(shape, F32, tag=tag + "e", name=tag + "e")
                for _ in range(iters):
                    nc.vector.tensor_tensor(out=e, in0=y, in1=y, op=AL.mult)
                    nc.vector.tensor_tensor(out=e, in0=e, in1=t, op=AL.mult)
                    nc.vector.tensor_scalar(out=e, in0=e, scalar1=-0.5,
                                            scalar2=1.5, op0=AL.mult,
                                            op1=AL.add)
                    nc.vector.tensor_tensor(out=y, in0=y, in1=e, op=AL.mult)
                return y

            def quant_scales(pool, am, ssq, dk, tag):
                rec = pool.tile([128, 1], F32, tag=tag + "rec", name=tag + "rec")
                nc.vector.reciprocal(out=rec, in_=am)
                q127 = pool.tile([128, 1], F32, tag=tag + "q", name=tag + "q")
                nc.vector.tensor_scalar(out=q127, in0=rec, scalar1=127.0,
                                        scalar2=None, op0=AL.mult)
                rs = newton2(pool, ssq, 1.0 / dk, 1e-8, [128, 1], magic1,
                             tag + "n", iters=1)
                dq = pool.tile([128, 1], F32, tag=tag + "dq", name=tag + "dq")
                nc.vector.tensor_tensor(out=dq, in0=am, in1=rs, op=AL.mult)
                nc.vector.tensor_scalar(out=dq, in0=dq, scalar1=1.0 / 127.0,
                                        scalar2=None, op0=AL.mult)
                return q127, dq

            def round_transpose(pool, src, q_ap, dst_bf, t_idx, nblk, tag):
                """round(src*q) -> bf16 ints, transpose 128-blocks into
                dst_bf[:, g4+jj, 128*t_idx:...]."""
                w = 128 * nblk
                t2 = pool.tile([128, w], F32, tag=tag + "t2", name=tag + "t2")
                nc.scalar.activation(out=t2, in_=src, func=AF.Copy,
                                     scale=q_ap, bias=C_MAGIC)
                kq = pool.tile([128, w], BF, tag=tag + "kq", name=tag + "kq")
                nc.vector.tensor_scalar(out=kq, in0=t2, scalar1=C_MAGIC,
                                        scalar2=None, op0=AL.subtract)
                for g4 in range(0, nblk, 4):
                    nb = min(4, nblk - g4)
                    tp = ps.tile([128, 512], BF, tag="tp", name="tp", bufs=2)
                    for jj in range(nb):
                        nc.tensor.transpose(
                            tp[:, 128 * jj:128 * (jj + 1)],
                            kq[:, 128 * (g4 + jj):128 * (g4 + jj + 1)],
                            identb)
                    for jj in range(nb):
                        dst = dst_bf[:, g4 + jj, 128 * t_idx:128 * (t_idx + 1)]
                        if jj % 2 == 0:
                            nc.scalar.copy(out=dst,
                                           in_=tp[:, 128 * jj:128 * (jj + 1)])
                        else:
                            nc.vector.tensor_copy(
                                out=dst, in_=tp[:, 128 * jj:128 * (jj + 1)])

            def bcast_row(pool, row_ap, bname, plus1=False, lhs=None):
                tt = pool.tile([128, D], F32, tag=bname, name=bname, bufs=1)
                for ch in range(0, D, 512):
                    pb = pbc()
                    nc.tensor.matmul(pb, ones_row if lhs is None else lhs,
                                     row_ap[:, ch:ch + 512], start=True,
                                     stop=True)
                    if plus1:
                        nc.scalar.activation(out=tt[:, ch:ch + 512], in_=pb,
                                             func=AF.Identity, bias=1.0)
                    else:
                        nc.scalar.copy(out=tt[:, ch:ch + 512], in_=pb)
                return tt

            def bcast_prow(pool, pidx, bname, plus1=False):
                """Broadcast adaln param row pidx (from prow_d) to [128, D]."""
                nc.scalar.dma_start(
                    out=prowc,
                    in_=prow_d[D * pidx:D * (pidx + 1)].rearrange(
                        "(one d) -> one d", one=1))
                return bcast_row(pool, prowc, bname, plus1=plus1)

            pCM = tc.alloc_tile_pool(name="pCM", bufs=1)
            x2qT = pCM.tile([128, 8, D], BF, tag="x2qT", name="x2qT")
            pWO = tc.alloc_tile_pool(name="pWO", bufs=1)
            wg_sb = pWO.tile([128, 8, D], BF, tag="wg", name="wg")
            wo_sb = pWO.tile([128, 8, D], BF, tag="wo", name="wo")
            xqT = pWO.tile([128, 8, D], BF, tag="xqT", name="xqT")
            gnr_sb = pWO.tile([1, D], F32, tag="gnr")
            nc.sync.dma_start(out=gnr_sb, in_=gnr[:, :])
            pHA = tc.alloc_tile_pool(name="pHA", bufs=1)
            ha = pHA.tile([128, 8, TOK], F32, tag="ha", name="ha")
            cam = pHA.tile([128, 8, CAMW], BF, tag="cam", name="cam")
            pL2 = tc.alloc_tile_pool(name="pL2", bufs=1)
            wi_sb = pL2.tile([128, 8, D], BF, tag="wi", name="wi")
            wf_sb = pL2.tile([128, 8, D], BF, tag="wf", name="wf")
            dqrow = pL2.tile([1, D], F32, tag="dqrow")
            Sb_i = pL2.tile([128, D], F32, tag="Sbi")
            Sb_f = pL2.tile([128, D], F32, tag="Sbf")

            # ---------------- prologue: prefetch + adaln ----------------
            pP = tc.alloc_tile_pool(name="pP", bufs=1)
            c_sb = pP.tile([128, 8, B], F32, tag="csb")
            nc.sync.dma_start(out=c_sb, in_=c_cols[:, :, :])
            adw_ch = []
            for jh in range(2):
                adw_h = pP.tile([128, 4, 768], F32, tag="adw", bufs=2)
                for jq in range(4):
                    nc.sync.dma_start(
                        out=adw_h[:, jq, :],
                        in_=adw_sl[512 * jh + 128 * jq:
                                   512 * jh + 128 * (jq + 1), :].rearrange(
                            "(a p) q -> p a q", p=128)[:, 0, :])
                adw_ch.append(adw_h)
            for (wsb, wdr) in ((wi_sb, wiT), (wf_sb, wfT)):
                for jh in range(2):
                    nc.gpsimd.dma_start(
                        out=wsb[:, 4 * jh:4 * (jh + 1), :],
                        in_=wdr[512 * jh:512 * (jh + 1), :].rearrange(
                            "(a p) q -> p a q", p=128))
            adb_sb = pP.tile([8, 768], F32, tag="adb")
            nc.sync.dma_start(out=adb_sb, in_=adb8[:, :])
            bm_sb = pP.tile([32, 8], F32, tag="bm")
            nc.sync.dma_start(out=bm_sb, in_=bmask_all[:, :])
            for (wsb, wdr) in ((wg_sb, wgT), (wo_sb, woT)):
                for jh in range(2):
                    nc.gpsimd.dma_start(
                        out=wsb[:, 4 * jh:4 * (jh + 1), :],
                        in_=wdr[512 * jh:512 * (jh + 1), :].rearrange(
                            "(a p) q -> p a q", p=128))

            # silu(c) = c * sigmoid(c) (keeps the sigmoid act table)
            csg = pP.tile([128, 8, B], F32, tag="csg")
            nc.scalar.activation(out=csg, in_=c_sb, func=AF.Sigmoid)
            cs_sb = pP.tile([128, 8, B], F32, tag="cssb")
            nc.vector.tensor_tensor(out=cs_sb, in0=c_sb, in1=csg, op=AL.mult)
            psA = pbc((B, 512))
            psB = pbc((B, 256))
            for j in range(8):
                aw = adw_ch[j // 4][:, j % 4, :]
                nc.tensor.matmul(psA, cs_sb[:, j, :], aw[:, 0:512],
                                 start=(j == 0), stop=(j == 7))
                nc.tensor.matmul(psB, cs_sb[:, j, :], aw[:, 512:768],
                                 start=(j == 0), stop=(j == 7))
            ad_sb = pP.tile([B, 768], F32, tag="adsb")
            nc.scalar.copy(out=ad_sb[:, 0:512], in_=psA)
            nc.scalar.copy(out=ad_sb[:, 512:768], in_=psB)
            nc.sync.dma_start(out=cc1_in[:, :], in_=ad_sb)
            nc.gpsimd.collective_compute(
                "AllGather", AL.bypass, ins=[cc1_in[:]], outs=[cc1_out[:]],
                replica_groups=RG1)

            # ---- param extraction: emitted early, runs when cc1 lands.
            # scalar/gpsimd queues so A1's sync-DMAs + DVE work don't block.
            ag_all = pP.tile([32, 768], F32, tag="agall", bufs=1)
            nc.scalar.dma_start(out=ag_all, in_=cc1_out[:, :])
            ps1 = pbc((8, 512))
            ps2 = pbc((8, 256))
            nc.tensor.matmul(ps1, bm_sb, ag_all[:, 0:512],
                             start=True, stop=True)
            nc.tensor.matmul(ps2, bm_sb, ag_all[:, 512:768],
                             start=True, stop=True)
            sel_raw = pP.tile([8, 768], F32, tag="selr", bufs=1)
            nc.scalar.copy(out=sel_raw[:, 0:512], in_=ps1)
            nc.scalar.copy(out=sel_raw[:, 512:768], in_=ps2)
            sel_sb = pP.tile([8, 768], F32, tag="sel", bufs=1)
            nc.gpsimd.tensor_tensor(out=sel_sb, in0=sel_raw, in1=adb_sb,
                                    op=AL.add)
            nc.scalar.dma_start(
                out=prow_d[:].rearrange("(a q) -> a q", a=8), in_=sel_sb)
            pP.release()

            # ======== phase A: LN1 (overlaps cc1) + modulate + quant ======
            rstdA = cst.tile([128, 8], F32, tag="rstdA")
            nmrA = cst.tile([128, 8], F32, tag="nmrA")
            with tc.tile_pool(name="pA", bufs=2) as pA:
                for t in range(8):
                    xt = pA.tile([128, D], F32, tag="xt", bufs=2)
                    nc.sync.dma_start(out=xt,
                                      in_=x_sl[128 * t:128 * (t + 1), :])
                    st = pA.tile([128, 2, 6], F32, tag="bst")
                    xr = xt.rearrange("p (s d) -> p s d", s=2)
                    for s2 in range(2):
                        nc.vector.bn_stats(out=st[:, s2, :], in_=xr[:, s2, :])
                    mv = pA.tile([128, 2], F32, tag="bmv")
                    nc.vector.bn_aggr(out=mv, in_=st)
                    rst = newton2(pA, mv[:, 1:2], 1.0, 1e-6, [128, 1],
                                  magic1, "rA")
                    nc.vector.tensor_copy(out=rstdA[:, t:t + 1], in_=rst)
                    nm = pA.tile([128, 1], F32, tag="nmA")
                    nc.vector.tensor_tensor(out=nm, in0=mv[:, 0:1], in1=rst,
                                            op=AL.mult)
                    nc.vector.tensor_scalar(out=nmrA[:, t:t + 1], in0=nm,
                                            scalar1=-1.0, scalar2=None,
                                            op0=AL.mult)

                B_sh1 = bcast_prow(pA, 0, "Bsh1")
                B_sc1 = bcast_prow(pA, 1, "Bsc1", plus1=True)

                # ---- modulate + quant + round + transpose, per tile ----
                for t in range(8):
                    xt = pA.tile([128, D], F32, tag="xt", bufs=2)
                    nc.sync.dma_start(out=xt,
                                      in_=x_sl[128 * t:128 * (t + 1), :])
                    mo = pA.tile([128, D], F32, tag="mo", bufs=2)
                    nc.scalar.activation(out=mo, in_=xt, func=AF.Identity,
                                         scale=rstdA[:, t:t + 1],
                                         bias=nmrA[:, t:t + 1])
                    nc.vector.tensor_tensor(out=mo, in0=mo, in1=B_sc1,
                                            op=AL.mult)
                    nc.vector.tensor_tensor(out=mo, in0=mo, in1=B_sh1,
                                            op=AL.add)
                    am = pA.tile([128, 1], F32, tag="amA")
                    nc.vector.tensor_reduce(out=am, in_=mo, axis=AX.X,
                                            op=AL.max,
                                            apply_absolute_value=True)
                    ssq = pA.tile([128, 1], F32, tag="ssA")
                    scr = pA.tile([128, D], F32, tag="rAt2", bufs=2)
                    nc.scalar.activation(out=scr, in_=mo, func=AF.Square,
                                         accum_out=ssq)
                    q127, dq = quant_scales(pA, am, ssq, D, "qA")
                    nc.vector.tensor_scalar(out=dqAg[:, t:t + 1], in0=dq,
                                            scalar1=float(iw["g"]),
                                            scalar2=None, op0=AL.mult)
                    pdq = pbc((1, 128))
                    nc.tensor.transpose(pdq, dq, identf)
                    nc.vector.tensor_copy(
                        out=dqrow[:, 128 * t:128 * (t + 1)], in_=pdq)
                    round_transpose(pA, mo, q127, xqT, t, 8, "rA")
                    if t == 3 or t == 7:
                        ch = 0 if t == 3 else 512
                        pb = pbc()
                        nc.tensor.matmul(pb, oi, dqrow[:, ch:ch + 512],
                                         start=True, stop=True)
                        nc.scalar.copy(out=Sb_i[:, ch:ch + 512], in_=pb)
                        pb2 = pbc()
                        nc.tensor.matmul(pb2, of, dqrow[:, ch:ch + 512],
                                         start=True, stop=True)
                        nc.scalar.copy(out=Sb_f[:, ch:ch + 512], in_=pb2)

            # ======== phase B: i/f matmuls + scan ========
            with tc.tile_pool(name="pB", bufs=2) as pB:
                for m in range(8):
                    ft = pB.tile([128, TOK], F32, tag="ft", bufs=2)
                    it = pB.tile([128, TOK], F32, tag="it", bufs=2)
                    for ck in (0, 512):
                        pf = pmm()
                        for j in range(8):
                            nc.tensor.matmul(pf, wf_sb[:, j, 128 * m:128 * (m + 1)],
                                             xqT[:, j, ck:ck + 512],
                                             start=(j == 0), stop=(j == 7))
                        pi = pmm()
                        for j in range(8):
                            nc.tensor.matmul(pi, wi_sb[:, j, 128 * m:128 * (m + 1)],
                                             xqT[:, j, ck:ck + 512],
                                             start=(j == 0), stop=(j == 7))
                        nc.vector.tensor_tensor(out=ft[:, ck:ck + 512], in0=pf,
                                                in1=Sb_f[:, ck:ck + 512],
                                                op=AL.mult)
                        nc.vector.tensor_tensor(out=it[:, ck:ck + 512], in0=pi,
                                                in1=Sb_i[:, ck:ck + 512],
                                                op=AL.mult)
                    sigf = pB.tile([128, TOK], F32, tag="sigf", bufs=2)
                    nc.scalar.activation(out=sigf, in_=ft, func=AF.Sigmoid)
                    sgi = pB.tile([128, TOK], F32, tag="sgi", bufs=2)
                    nc.scalar.activation(out=sgi, in_=it, func=AF.Sigmoid)
                    # omf = 1 - sigf (in place over ft); ifin = it*sgi*omf
                    nc.vector.tensor_scalar(out=ft, in0=sigf, scalar1=-1.0,
                                            scalar2=1.0, op0=AL.mult,
                                            op1=AL.add)
                    nc.gpsimd.tensor_tensor(out=it, in0=it, in1=sgi,
                                            op=AL.mult)
                    nc.gpsimd.tensor_tensor(out=it, in0=it, in1=ft,
                                            op=AL.mult)
                    nc.vector.tensor_tensor_scan(ha[:, m, :], sigf, it, 0.0,
                                                 op0=AL.mult, op1=AL.add)
                    nc.vector.tensor_tensor_scan(cam[:, m, :],
                                                 sigf[:, 0:CAMW],
                                                 sigf[:, 0:CAMW], 1.0,
                                                 op0=AL.mult, op1=AL.bypass)
                    nc.vector.tensor_copy(out=lc[:, m:m + 1],
                                          in_=ha[:, m, TOK - 1:TOK])
                plc = pbc((8, 128))
                nc.tensor.transpose(plc, lc, identf)
                lcT = pB.tile([8, 128], F32, tag="lcT", bufs=1)
                nc.vector.tensor_copy(out=lcT, in_=plc)
                nc.sync.dma_start(
                    out=cc2_in[:].rearrange("(a q) -> a q", a=8), in_=lcT)
                nc.gpsimd.collective_compute(
                    "AllGather", AL.bypass, ins=[cc2_in[:]], outs=[cc2_out[:]],
                    replica_groups=RG1)

            pL2.release()

            # ======== phase OC: per-tile attention-out + residual + LN2 ===
            # tile order [2..7, 0, 1]: tiles >= 2 are outside the carry
            # window so they proceed while the AllGather is in flight.
            with tc.tile_pool(name="pOC", bufs=2) as pOC:
                B_gn = bcast_row(pOC, gnr_sb, "Bgn")
                B_g1 = bcast_prow(pOC, 2, "Bg1")
                B_sh2 = bcast_prow(pOC, 3, "Bsh2")
                B_sc2 = bcast_prow(pOC, 4, "Bsc2", plus1=True)
                carr = pOC.tile([128, 8], F32, tag="carr", bufs=1)
                for t in [2, 3, 4, 5, 6, 7, 0, 1]:
                    if t == 0:
                        # consume cc2: carry per channel block, then fix the
                        # first CAMW tokens of ha
                        ag2 = pOC.tile([N_CORES, D], F32, tag="ag2", bufs=1)
                        nc.sync.dma_start(out=ag2, in_=cc2_out[:, :])
                        for m in range(8):
                            pc = pbc((128, 1))
                            nc.tensor.matmul(pc, ag2[:, 128 * m:128 * (m + 1)],
                                             mask_sb, start=True, stop=True)
                            nc.vector.tensor_copy(out=carr[:, m:m + 1],
                                                  in_=pc)
                        for m in range(8):
                            nc.vector.scalar_tensor_tensor(
                                out=ha[:, m, 0:CAMW], in0=cam[:, m, :],
                                scalar=carr[:, m:m + 1], in1=ha[:, m, 0:CAMW],
                                op0=AL.mult, op1=AL.add)
                    # transpose h column-block t (all m) into hT_t
                    hTt = pOC.tile([128, D], F32, tag="hTt", bufs=2)
                    for g4 in range(0, 8, 4):
                        tpf = pbc()
                        for jj in range(4):
                            m = g4 + jj
                            nc.tensor.transpose(
                                tpf[:, 128 * jj:128 * (jj + 1)],
                                ha[:, m, 128 * t:128 * (t + 1)], identf)
                        for jj in range(4):
                            m = g4 + jj
                            dst = hTt[:, 128 * m:128 * (m + 1)]
                            if jj % 2 == 0:
                                nc.scalar.copy(
                                    out=dst,
                                    in_=tpf[:, 128 * jj:128 * (jj + 1)])
                            else:
                                nc.vector.tensor_copy(
                                    out=dst,
                                    in_=tpf[:, 128 * jj:128 * (jj + 1)])
                    # g projection for this tile
                    gst = pOC.tile([128, D], F32, tag="gst", bufs=2)
                    for ck in (0, 512):
                        pg = pmm()
                        for j in range(8):
                            nc.tensor.matmul(pg, xqT[:, j, 128 * t:128 * (t + 1)],
                                             wg_sb[:, j, ck:ck + 512],
                                             start=(j == 0), stop=(j == 7))
                        sil = pOC.tile([128, 512], F32, tag="silg", bufs=2)
                        nc.scalar.activation(out=sil, in_=pg, func=AF.Silu,
                                             scale=dqAg[:, t:t + 1])
                        nc.vector.tensor_tensor(out=gst[:, ck:ck + 512],
                                                in0=sil,
                                                in1=B_gn[:, ck:ck + 512],
                                                op=AL.mult)
                    # GN + gate + quantize o
                    sq = pOC.tile([128, D], F32, tag="big", bufs=2)
                    nc.gpsimd.tensor_tensor(out=sq, in0=hTt, in1=hTt,
                                            op=AL.mult)
                    msh = pOC.tile([128, NH], F32, tag="msh", bufs=2)
                    nc.vector.tensor_reduce(
                        out=msh, in_=sq.rearrange("p (h d) -> p h d", h=NH),
                        axis=AX.X, op=AL.add)
                    rsH = newton2(pOC, msh, 1.0 / HD, 1e-5, [128, NH],
                                  magic16, "rH")
                    rb = bass.AP(tensor=rsH.tensor, offset=rsH.offset,
                                 ap=[rsH.ap[0], [1, NH], [0, HD]])
                    oa = pOC.tile([128, D], F32, tag="big", bufs=2)
                    nc.vector.tensor_tensor(
                        out=oa.rearrange("p (h d) -> p h d", h=NH),
                        in0=hTt.rearrange("p (h d) -> p h d", h=NH),
                        in1=rb, op=AL.mult)
                    nc.vector.tensor_tensor(out=oa, in0=oa, in1=gst,
                                            op=AL.mult)
                    am = pOC.tile([128, 1], F32, tag="amO")
                    nc.vector.tensor_reduce(out=am, in_=oa, axis=AX.X,
                                            op=AL.max,
                                            apply_absolute_value=True)
                    ssq = pOC.tile([128, 1], F32, tag="ssO")
                    scr = pOC.tile([128, D], F32, tag="rtt2", bufs=2)
                    nc.scalar.activation(out=scr, in_=oa, func=AF.Square,
                                         accum_out=ssq)
                    q127, dq = quant_scales(pOC, am, ssq, D, "qO")
                    dqO = pOC.tile([128, 1], F32, tag="dqO", bufs=2)
                    nc.vector.tensor_scalar(out=dqO, in0=dq,
                                            scalar1=float(iw["o"]),
                                            scalar2=None, op0=AL.mult)
                    oqTt = pOC.tile([128, 8, 128], BF, tag="oqTt", bufs=2)
                    round_transpose(pOC, oa, q127, oqTt, 0, 8, "rt")
                    # wo matmul + residual + LN2 + modulate + quantize
                    x2 = pOC.tile([128, D], F32, tag="x2", bufs=2)
                    nc.sync.dma_start(out=x2,
                                      in_=x_sl[128 * t:128 * (t + 1), :])
                    xn = pOC.tile([128, D], F32, tag="xn", bufs=2)
                    for ck in (0, 512):
                        at = pmm()
                        for j in range(8):
                            nc.tensor.matmul(at, oqTt[:, j, :],
                                             wo_sb[:, j, ck:ck + 512],
                                             start=(j == 0), stop=(j == 7))
                        ga = pOC.tile([128, 512], F32, tag="ga", bufs=2)
                        nc.vector.scalar_tensor_tensor(
                            out=ga, in0=at, scalar=dqO,
                            in1=B_g1[:, ck:ck + 512], op0=AL.mult,
                            op1=AL.mult)
                        nc.gpsimd.tensor_tensor(out=xn[:, ck:ck + 512],
                                                in0=ga, in1=x2[:, ck:ck + 512],
                                                op=AL.add)
                    nc.sync.dma_start(out=xnew_d[128 * t:128 * (t + 1), :],
                                      in_=xn)
                    st = pOC.tile([128, 2, 6], F32, tag="bst2")
                    xr = xn.rearrange("p (s d) -> p s d", s=2)
                    for s2 in range(2):
                        nc.vector.bn_stats(out=st[:, s2, :], in_=xr[:, s2, :])
                    mv = pOC.tile([128, 2], F32, tag="bmv2")
                    nc.vector.bn_aggr(out=mv, in_=st)
                    rst = newton2(pOC, mv[:, 1:2], 1.0, 1e-6, [128, 1],
                                  magic1, "rC")
                    nm = pOC.tile([128, 1], F32, tag="nmC")
                    nc.vector.tensor_tensor(out=nm, in0=mv[:, 0:1], in1=rst,
                                            op=AL.mult)
                    nc.vector.tensor_scalar(out=nm, in0=nm, scalar1=-1.0,
                                            scalar2=None, op0=AL.mult)
                    u2 = pOC.tile([128, D], F32, tag="u2", bufs=2)
                    nc.scalar.activation(out=u2, in_=xn, func=AF.Identity,
                                         scale=rst, bias=nm)
                    nc.vector.tensor_tensor(out=u2, in0=u2, in1=B_sc2,
                                            op=AL.mult)
                    nc.vector.tensor_tensor(out=u2, in0=u2, in1=B_sh2,
                                            op=AL.add)
                    am2 = pOC.tile([128, 1], F32, tag="amC")
                    nc.vector.tensor_reduce(out=am2, in_=u2, axis=AX.X,
                                            op=AL.max,
                                            apply_absolute_value=True)
                    ssq2 = pOC.tile([128, 1], F32, tag="ssC")
                    scr2 = pOC.tile([128, D], F32, tag="rtt2", bufs=2)
                    nc.scalar.activation(out=scr2, in_=u2, func=AF.Square,
                                         accum_out=ssq2)
                    q127c, dqc = quant_scales(pOC, am2, ssq2, D, "qC")
                    nc.vector.tensor_scalar(out=dqCg[:, t:t + 1], in0=dqc,
                                            scalar1=float(iw["gate"]),
                                            scalar2=None, op0=AL.mult)
                    round_transpose(pOC, u2, q127c, x2qT, t, 8, "rt")

            pHA.release()
            pWO.release()

            # ======== phase D/E: MLP in two 512-token supertiles ========
            with tc.tile_pool(name="pM", bufs=2) as pM:
                B_g2 = bcast_prow(pM, 5, "Bg2")
                dw_sb = pM.tile([128, 32, D], BF, tag="dw", bufs=1)
                h2 = pM.tile([128, 4, MLP], F16, tag="h2", bufs=1)
                amDg = pM.tile([128, 4, 8], F32, tag="amDg", bufs=1)
                ssDg = pM.tile([128, 4, 8], F32, tag="ssDg", bufs=1)
                for s in range(2):
                    for g in range(8):
                        gwg = pM.tile([128, 8, 512], BF, tag="gwg", bufs=2)
                        gwy = pM.tile([128, 8, 512], BF, tag="gwy", bufs=2)
                        for ah in range(2):
                            nc.gpsimd.dma_start(
                                out=gwg[:, 4 * ah:4 * (ah + 1), :],
                                in_=gwT[512 * ah:512 * (ah + 1),
                                        512 * g:512 * (g + 1)].rearrange(
                                    "(a p) q -> p a q", p=128))
                            nc.gpsimd.dma_start(
                                out=gwy[:, 4 * ah:4 * (ah + 1), :],
                                in_=gwT[512 * ah:512 * (ah + 1),
                                        MLP + 512 * g:MLP + 512 * (g + 1)]
                                .rearrange("(a p) q -> p a q", p=128))
                        for tt in range(4):
                            t = 4 * s + tt
                            pg = pmm()
                            for j in range(8):
                                nc.tensor.matmul(
                                    pg, x2qT[:, j, 128 * t:128 * (t + 1)],
                                    gwg[:, j, :], start=(j == 0),
                                    stop=(j == 7))
                            py = pmm()
                            for j in range(8):
                                nc.tensor.matmul(
                                    py, x2qT[:, j, 128 * t:128 * (t + 1)],
                                    gwy[:, j, :], start=(j == 0),
                                    stop=(j == 7))
                            sil = pM.tile([128, 512], F32, tag="silm", bufs=2)
                            nc.scalar.activation(out=sil, in_=pg,
                                                 func=AF.Silu,
                                                 scale=dqCg[:, t:t + 1])
                            h2s = h2[:, tt, 512 * g:512 * (g + 1)]
                            nc.vector.tensor_tensor(out=h2s, in0=sil, in1=py,
                                                    op=AL.mult)
                            nc.vector.tensor_reduce(
                                out=amDg[:, tt, g:g + 1], in_=h2s, axis=AX.X,
                                op=AL.max, apply_absolute_value=True)
                            scr = pM.tile([128, 512], F32, tag="sqD", bufs=1)
                            nc.scalar.activation(
                                out=scr, in_=h2s, func=AF.Square,
                                accum_out=ssDg[:, tt, g:g + 1])
                        if s == 0:
                            nc.gpsimd.dma_start(
                                out=dw_sb[:, 4 * g:4 * (g + 1), :],
                                in_=dwT[512 * g:512 * (g + 1), :]
                                .rearrange("(a p) q -> p a q", p=128))
                    # ---- round+transpose / down-proj, 2-stage skew ----
                    def mlp_round(tt):
                        t = 4 * s + tt
                        am = pM.tile([128, 1], F32, tag="amD")
                        nc.vector.tensor_reduce(out=am, in_=amDg[:, tt, :],
                                                axis=AX.X, op=AL.max)
                        ssq = pM.tile([128, 1], F32, tag="ssD")
                        nc.vector.tensor_reduce(out=ssq, in_=ssDg[:, tt, :],
                                                axis=AX.X, op=AL.add)
                        xn3 = pM.tile([128, D], F32, tag="xn3", bufs=2)
                        nc.sync.dma_start(
                            out=xn3, in_=xnew_d[128 * t:128 * (t + 1), :])
                        q127, dq = quant_scales(pM, am, ssq, MLP, "qD")
                        nc.vector.tensor_scalar(out=dqD8[:, t:t + 1], in0=dq,
                                                scalar1=float(iw["down"]),
                                                scalar2=None, op0=AL.mult)
                        h2qT = pM.tile([128, 32, 128], BF, tag="h2qT", bufs=2)
                        for qc in range(4):
                            round_transpose(
                                pM, h2[:, tt, 1024 * qc:1024 * (qc + 1)],
                                q127, h2qT[:, 8 * qc:8 * (qc + 1), :],
                                0, 8, "rD")
                        return h2qT, xn3

                    def mlp_down(tt, h2qT, xn3):
                        t = 4 * s + tt
                        outt = pM.tile([128, D], F32, tag="outt", bufs=2)
                        for ck in (0, 512):
                            pdn = pmm()
                            for j2 in range(32):
                                nc.tensor.matmul(pdn, h2qT[:, j2, :],
                                                 dw_sb[:, j2, ck:ck + 512],
                                                 start=(j2 == 0),
                                                 stop=(j2 == 31))
                            gd = pM.tile([128, 512], F32, tag="gd", bufs=1)
                            nc.vector.scalar_tensor_tensor(
                                out=gd, in0=pdn, scalar=dqD8[:, t:t + 1],
                                in1=B_g2[:, ck:ck + 512], op0=AL.mult,
                                op1=AL.mult)
                            nc.vector.tensor_tensor(
                                out=outt[:, ck:ck + 512], in0=gd,
                                in1=xn3[:, ck:ck + 512], op=AL.add)
                        nc.sync.dma_start(
                            out=out_sl[128 * t:128 * (t + 1), :], in_=outt)

                    prevr = (0, mlp_round(0))
                    for tt in range(4):
                        nxt = (tt + 1, mlp_round(tt + 1)) if tt < 3 else None
                        pt, args = prevr
                        mlp_down(pt, *args)
                        prevr = nxt

            pCM.release()

    nc.finalize()
    return nc


@functools.lru_cache(maxsize=2)
def _build_cached(iw_items):
    return _build(dict(iw_items))


def kernel(x, c, adaln_w, adaln_b, wi, wf, wg, gnorm_w, wo, gate_w, down_w):
    x = np.ascontiguousarray(np.asarray(x, dtype=np.float32))
    c = np.ascontiguousarray(np.asarray(c, dtype=np.float32))
    adaln_w = np.asarray(adaln_w, dtype=np.float32)
    adaln_b = np.asarray(adaln_b, dtype=np.float32)
    gnorm_w = np.asarray(gnorm_w, dtype=np.float32)

    mi, iwi = _quant_w(np.asarray(wi, dtype=np.float32))
    mf, iwf = _quant_w(np.asarray(wf, dtype=np.float32))
    mg, iwg = _quant_w(np.asarray(wg, dtype=np.float32))
    mo, iwo = _quant_w(np.asarray(wo, dtype=np.float32))
    mgate, iwgate = _quant_w(np.asarray(gate_w, dtype=np.float32))
    mdown, iwdown = _quant_w(np.asarray(down_w, dtype=np.float32))

    iw = {"i": float(iwi), "f": float(iwf), "g": float(iwg), "o": float(iwo),
          "gate": float(iwgate), "down": float(iwdown)}
    nc = _build_cached(tuple(sorted(iw.items())))

    wiT_h = np.ascontiguousarray(mi.T)
    wfT_h = np.ascontiguousarray(mf.T)
    wgT_h = np.ascontiguousarray(mg.T)
    woT_h = np.ascontiguousarray(mo.T)
    gwT_h = np.ascontiguousarray(mgate.T)
    dwT_h = np.ascontiguousarray(mdown.T)
    adwT = np.ascontiguousarray(adaln_w.T)          # [D, 6D] f32
    adb8_h = np.ascontiguousarray(adaln_b.reshape(8, 768))
    gnr_h = np.ascontiguousarray(np.tile(gnorm_w, NH)[None, :])
    c_cols_h = np.ascontiguousarray(
        c.T.reshape(8, 128, B).transpose(1, 0, 2))   # [128, 8, B]

    in_maps = []
    for core in range(N_CORES):
        b, half = core // 2, core % 2
        bmask = np.zeros((32, 8), np.float32)
        for r in range(8):
            bmask[4 * r + b, r] = 1.0
        m8 = np.zeros((8, 1), np.float32)
        if half == 1:
            m8[core - 1, 0] = 1.0

        in_maps.append({
            "x_sl": np.ascontiguousarray(x[b, half * TOK:(half + 1) * TOK, :]),
            "c_cols": c_cols_h,
            "adw_sl": np.ascontiguousarray(
                adwT[:, 768 * core:768 * (core + 1)]),
            "adb8": adb8_h,
            "bmask_all": bmask,
            "mask8": m8,
            "gnr": gnr_h,
            "wiT": wiT_h, "wfT": wfT_h, "wgT": wgT_h, "woT": woT_h,
            "gwT": gwT_h, "dwT": dwT_h,
        })

    res = run_bass_kernel_spmd(nc, in_maps, core_ids=list(range(N_CORES)))
    out = np.zeros((B, T, D), np.float32)
    for core in range(N_CORES):
        b, half = core // 2, core % 2
        out[b, half * TOK:(half + 1) * TOK, :] = res.results[core]["out_sl"]
    return out
